# revision 8
# baseline (speedup 1.0000x reference)
"""KNN k-th-nearest-distance kernel for Trainium2 (8 NeuronCores).

Math: reference computes, per query row n, the k-th smallest of
dist[n,m] = sqrt(max(||zn||^2 + ||rn||^2 - 2 zn.rn, 1e-12)) over m,
with zn/rn the L2-normalized rows of z [2048,128] / ref [50000,128].
Since ||zn||^2 == ||rn||^2 == 1 (to fp32 rounding), dist is a
monotonically *decreasing* function of s = zn.rn, so the k-th smallest
distance corresponds to the k-th *largest* inner product s.

Device work, v2 (queries sharded 8-way, ref *prep* sharded 8-way):
  Phase A (per core, own 1/8 slice of ref, zero-padded to 6400 rows):
    normalize rows in fp32, cast to bf16, PE-transpose -> rnT slice
    [128, 6400] bf16, then AllGather the slices (4 pipelined pieces)
    so every core sees rnT for all 50000 rows at 1/8 the prep cost.
  Phase B: s_raw = z @ rnT via bf16 PE matmul (z unnormalized: the
    positive per-row scale 1/|z_n| does not change per-row ordering),
    1024-wide PSUM windows, DVE max8 per window (half the windows are
    first evacuated to SBUF by the Scalar engine to split the scan
    cost across two engines).
  Final: max8 + match_replace + max8 over window candidates -> top-16
    raw scores per query row.
Host: divide by |z_n|, dist = sqrt(max(2 - 2*s, 1e-12)), pick column k.

Zero-padded ref rows: 1/norm is computed as 1/max(sqrt(ssq), 1e-20),
so padded rows stay exactly 0 and their sims are 0 -- far below the
top-11 sims (~ +0.37|z|) for randn data, so they never pollute top-k.
"""

import os
import sys

sys.path.insert(0, "/opt/trn_rl_repo")

from contextlib import ExitStack

import numpy as np


def _install_ntff_hook_shim():
    """The agent image's antenv lacks axon_hooks, so trace=True degrades.
    Recreate the hook module + the ctypes NTFF driver (mirrors
    trn_agent_boot.trn_boot._ntff_profile_via_ctypes)."""
    import contextlib
    import ctypes
    import types

    if "antenv.axon_hooks" in sys.modules:
        return
    so_path = "/opt/axon/libaxon_pjrt.so"
    mod = types.ModuleType("antenv.axon_hooks")
    state = {"hook": None}

    def set_axon_ntff_profile_hook(h):
        state["hook"] = h

    def get_axon_ntff_profile_hook():
        return state["hook"]

    mod.set_axon_ntff_profile_hook = set_axon_ntff_profile_hook
    mod.get_axon_ntff_profile_hook = get_axon_ntff_profile_hook
    sys.modules["antenv.axon_hooks"] = mod

    try:
        lib = ctypes.CDLL(so_path)
        if not hasattr(lib, "axon_start_nrt_profile"):
            return
        lib.axon_start_nrt_profile.argtypes = [
            ctypes.POINTER(ctypes.c_int64),
            ctypes.c_size_t,
        ]
        lib.axon_start_nrt_profile.restype = ctypes.c_int64
        lib.axon_stop_nrt_profile.argtypes = [ctypes.c_char_p]
        lib.axon_stop_nrt_profile.restype = ctypes.c_int64

        @contextlib.contextmanager
        def _hook(output_dir, device_ids):
            import jax

            jax.devices()
            if device_ids:
                ids = (ctypes.c_int64 * len(device_ids))(*device_ids)
                rc = lib.axon_start_nrt_profile(ids, len(device_ids))
            else:
                rc = lib.axon_start_nrt_profile(None, 0)
            if rc != 0:
                raise RuntimeError(f"axon_start_nrt_profile rc={rc}")
            try:
                yield
            finally:
                n = lib.axon_stop_nrt_profile(str(output_dir).encode())
                print(f"ntff profile: {n} file(s) written to {output_dir}")

        state["hook"] = _hook
    except Exception:
        pass


_install_ntff_hook_shim()

import concourse.bacc as bacc
import concourse.bass as bass
import concourse.tile as tile
from concourse import mybir
from concourse.bass_utils import run_bass_kernel_spmd

N, M, D = 2048, 50000, 128
NCORES = 8
NPC = N // NCORES  # 256 queries per core
SLICE = 6400  # ref rows prepped per core (core 7 zero-padded)
# AllGather pieces (columns of the transposed slice) for pipelining
PIECES = (2048, 2048, 2048, 256)
# Phase-B max8 windows within each piece
WINS = {2048: (1024, 1024), 256: (256,)}
NWIN_SLICE = sum(len(WINS[p]) for p in PIECES)  # 5 windows per slice
TOPW = NCORES * NWIN_SLICE * 8  # candidate width per query row

F32 = mybir.dt.float32
BF16 = mybir.dt.bfloat16

_CACHE = {}
LAST_RESULTS = None


def _build():
    nblk = NPC // 128

    nc = bacc.Bacc(
        "TRN2", target_bir_lowering=False, debug=False, num_devices=NCORES
    )
    z_d = nc.dram_tensor("z", [NPC, D], F32, kind="ExternalInput")
    ref_d = nc.dram_tensor("refsl", [SLICE, D], F32, kind="ExternalInput")
    out_d = nc.dram_tensor("top16", [NPC, 16], F32, kind="ExternalOutput")
    ident_d = nc.inline_tensor(
        np.eye(128, dtype=np.float32).astype(np.dtype("bfloat16"))
        if False
        else np.eye(128, dtype=np.float32),
        name="ident",
    )

    z = z_d.ap()
    ref = ref_d.ap()
    out16 = out_d.ap()

    with tile.TileContext(nc) as tc, ExitStack() as ctx:
        const_pool = ctx.enter_context(tc.tile_pool(name="const", bufs=1))
        zpool = ctx.enter_context(tc.tile_pool(name="zp", bufs=1))
        persist = ctx.enter_context(tc.tile_pool(name="persist", bufs=1))
        cand_pool = ctx.enter_context(tc.tile_pool(name="cand", bufs=1))
        dram = ctx.enter_context(tc.tile_pool(name="dram", bufs=1, space="DRAM"))
        fin_pool = ctx.enter_context(tc.tile_pool(name="fin", bufs=1))

        identf = const_pool.tile([128, 128], F32, name="identf")
        nc.sync.dma_start(identf[:, :], ident_d.ap()[:, :])
        identb = const_pool.tile([128, 128], BF16, name="identb")
        nc.vector.tensor_copy(identb[:, :], identf[:, :])

        # ---- z: [256,128] f32 -> znT[b] [128,128] bf16 (PE transpose) ----
        znT = []
        with tc.tile_pool(name="zps", bufs=1, space="PSUM") as zpsum:
            for b in range(nblk):
                zt = zpool.tile([128, D], F32, tag="zload")
                nc.sync.dma_start(zt[:, :], z[b * 128 : (b + 1) * 128, :])
                ztb = zpool.tile([128, D], BF16, tag="zloadb")
                nc.vector.tensor_copy(ztb[:, :], zt[:, :])
                zp = zpsum.tile([128, 128], BF16, tag="zpsum")
                nc.tensor.transpose(zp[:, :], ztb[:, :], identb[:, :])
                zs = persist.tile([128, 128], BF16, tag=f"znT{b}", name=f"znT{b}")
                nc.scalar.copy(zs[:, :], zp[:, :])
                znT.append(zs)

        cand = [
            cand_pool.tile([128, TOPW], F32, tag=f"cand{b}", name=f"cand{b}")
            for b in range(nblk)
        ]

        # ---- Phase A: normalize own slice, transpose, gather ----
        rnT_sl = persist.tile([128, SLICE], BF16, name="rnT_sl")
        gin = []
        gout = []
        for p, pw in enumerate(PIECES):
            gi = dram.tile([128, pw], BF16, tag=f"gin{p}", name=f"gin{p}")
            go = dram.tile(
                [NCORES, 128, pw], BF16, tag=f"gout{p}", name=f"gout{p}"
            )
            gin.append(gi)
            gout.append(go)

        with (
            tc.tile_pool(name="rload", bufs=3) as rload_pool,
            tc.tile_pool(name="sq", bufs=2) as sq_pool,
            tc.tile_pool(name="stat", bufs=3) as stat_pool,
            tc.tile_pool(name="rsc", bufs=3) as rsc_pool,
            tc.tile_pool(name="tps", bufs=2, space="PSUM") as tpsum_pool,
        ):
            col = 0  # column offset within rnT_sl
            for p, pw in enumerate(PIECES):
                ngrp = pw // 512 if pw >= 512 else 0
                grps = [512] * ngrp + ([pw % 512] if pw % 512 else [])
                pcol = 0  # column offset within the piece
                for gw in grps:
                    G = gw // 128
                    m0 = col + pcol
                    rl = rload_pool.tile([128, 4, 128], F32, tag="rload")
                    nc.sync.dma_start(
                        rl[:, :G, :],
                        ref[m0 : m0 + gw, :].rearrange(
                            "(g p) d -> p g d", p=128
                        ),
                    )
                    ssq = stat_pool.tile([128, 4], F32, tag="ssq")
                    for g in range(G):
                        sq = sq_pool.tile([128, 128], F32, tag="sq")
                        nc.vector.scalar_tensor_tensor(
                            out=sq[:, :],
                            in0=rl[:, g, :],
                            scalar=1.0,
                            in1=rl[:, g, :],
                            op0=mybir.AluOpType.mult,
                            op1=mybir.AluOpType.mult,
                            accum_out=ssq[:, g : g + 1],
                        )
                    sn = stat_pool.tile([128, 4], F32, tag="sn")
                    nc.scalar.sqrt(sn[:, :G], ssq[:, :G])
                    # clamp so zero-padded rows give 0*big = 0, not NaN
                    sc = stat_pool.tile([128, 4], F32, tag="sc")
                    nc.vector.tensor_scalar_max(sc[:, :G], sn[:, :G], 1e-20)
                    rq = stat_pool.tile([128, 4], F32, tag="rq")
                    nc.vector.reciprocal(rq[:, :G], sc[:, :G])

                    rsc = rsc_pool.tile([128, 4, 128], BF16, tag="rsc")
                    for g in range(G):
                        nc.vector.tensor_scalar_mul(
                            rsc[:, g, :], rl[:, g, :], rq[:, g : g + 1]
                        )
                    tp = tpsum_pool.tile([128, 512], BF16, tag="tps")
                    for g in range(G):
                        nc.tensor.transpose(
                            tp[:, g * 128 : (g + 1) * 128],
                            rsc[:, g, :],
                            identb[:, :],
                        )
                    nc.scalar.copy(
                        rnT_sl[:, m0 : m0 + gw], tp[:, :gw]
                    )
                    pcol += gw
                # piece done: bounce to DRAM and gather
                nc.sync.dma_start(
                    gin[p][:, :], rnT_sl[:, col : col + pw]
                )
                nc.gpsimd.collective_compute(
                    "AllGather",
                    mybir.AluOpType.bypass,
                    replica_groups=[list(range(NCORES))],
                    ins=[gin[p].opt()],
                    outs=[gout[p].opt()],
                )
                col += pw

        # ---- Phase B: matmuls over gathered rnT + windowed max8 ----
        widx = 0
        with (
            tc.tile_pool(name="mv", bufs=4) as mv_pool,
            tc.tile_pool(name="sp", bufs=2, space="PSUM") as spsum_pool,
            tc.tile_pool(name="sevac", bufs=3) as sevac_pool,
        ):
            for csrc in range(NCORES):
                for p, pw in enumerate(PIECES):
                    pcol = 0
                    for w in WINS[pw]:
                        mv = mv_pool.tile([128, 1024], BF16, tag="mv")
                        nc.sync.dma_start(
                            mv[:, :w],
                            gout[p][csrc, :, pcol : pcol + w],
                        )
                        for b in range(nblk):
                            sp = spsum_pool.tile(
                                [128, 1024], F32, tag=f"sp{b}"
                            )
                            for h in range(0, w, 512):
                                hw = min(512, w - h)
                                nc.tensor.matmul(
                                    sp[:, h : h + hw],
                                    znT[b][:, :],
                                    mv[:, h : h + hw],
                                    start=True,
                                    stop=True,
                                )
                            dst = cand[b][:, widx * 8 : (widx + 1) * 8]
                            if widx % 2 == 0:
                                nc.vector.max(dst, sp[:, :w])
                            else:
                                ev = sevac_pool.tile(
                                    [128, 1024], BF16, tag="ev"
                                )
                                nc.scalar.copy(ev[:, :w], sp[:, :w])
                                nc.vector.max(dst, ev[:, :w])
                        pcol += w
                        widx += 1

        # ---- Final: exact top-16 from candidates ----
        for b in range(nblk):
            t8a = fin_pool.tile([128, 8], F32, tag=f"t8a{b}")
            nc.vector.max(t8a[:, :], cand[b][:, :])
            cand2 = fin_pool.tile([128, TOPW], F32, tag=f"cand2{b}")
            nc.vector.match_replace(cand2[:, :], t8a[:, :], cand[b][:, :], -3.0)
            t8b = fin_pool.tile([128, 8], F32, tag=f"t8b{b}")
            nc.vector.max(t8b[:, :], cand2[:, :])
            nc.sync.dma_start(out16[b * 128 : (b + 1) * 128, 0:8], t8a[:, :])
            nc.sync.dma_start(out16[b * 128 : (b + 1) * 128, 8:16], t8b[:, :])

    nc.compile()
    return nc


def kernel(z, ref, k):
    global LAST_RESULTS
    z_np = np.ascontiguousarray(np.asarray(z, dtype=np.float32))
    ref_np = np.ascontiguousarray(np.asarray(ref, dtype=np.float32))
    kk = int(k)

    if "nc" not in _CACHE:
        _CACHE["nc"] = _build()
    nc = _CACHE["nc"]

    refp = np.zeros((NCORES * SLICE, D), dtype=np.float32)
    refp[:M] = ref_np
    in_maps = [
        {
            "z": np.ascontiguousarray(z_np[i * NPC : (i + 1) * NPC]),
            "refsl": np.ascontiguousarray(refp[i * SLICE : (i + 1) * SLICE]),
        }
        for i in range(NCORES)
    ]
    res = run_bass_kernel_spmd(nc, in_maps, core_ids=list(range(NCORES)))
    LAST_RESULTS = res
    top16 = np.concatenate([r["top16"] for r in res.results], axis=0)  # [N,16]

    znorm = np.sqrt(np.sum(z_np.astype(np.float32) ** 2, axis=1))  # [N]
    s = top16[:, kk] / znorm
    return np.sqrt(np.maximum(2.0 - 2.0 * s, 1e-12)).astype(np.float32)
